# revision 36
# baseline (speedup 1.0000x reference)
"""BankModulatedConv Trainium2 kernel.

Problem (per sample b of B=8):
  w = softmax(bank_request[b])                        # (16,)
  kern = sum_f w[f] * bank_weight[f]                  # (o, i, kh, kw) = (256, 256, 3, 3)
  kern *= (1 + style[b, i])                           # input-channel modulation
  kern *= rsqrt(sum_{i,kh,kw} kern^2 + 1e-8)          # per-o L2 demodulation
  y[b] = conv2d(x[b], kern, stride 1, SAME)           # (256, 64, 64)

Mapping: the work grid is 8 samples x 2 o-chunks = 16 units; core c takes
o-chunk (c % 2) for the sample pair (2*(c//2), 2*(c//2)+1).  This halves the
per-core filter-bank DMA (9.4 MB bf16 instead of 18.9 MB replicated) while
keeping every conv matmul at full M=128, and demodulation stays core-local
(the per-o L2 sum runs over (i, khw) which this core holds completely).

Per core:
  - bank ships host-rearranged as [ic(2), fp(8), i(128), j(2), khw(9),
    o_local(128)] bf16: each DMA row is 4608 contiguous bytes, and the
    khw-major column order means the conv lhsT slice for one (kh, kw) tap
    is a fully contiguous [128, 128] block (a strided lhsT fetch costs
    ~35 ns per matmul in exposed ldweights time).  Constants (I_128, ones)
    ride in a separate tiny [128, 257] DMA issued first.
  - mixing is split per f across engines to shorten the serial window:
    PE takes cols 0:768 of each 1152-col khw/o block (16 accumulated
    diag(w_f) matmuls into two PSUM slices per sample), DVE takes cols
    768:1152 as a scalar_tensor_tensor MAC chain.  Both are paced by the
    bank tile arrivals; bf16 is storage precision only (f32 accumulate).
  - style modulation (1+style[i]) is fused into the PSUM/acc -> bf16
    kernel copies on ScalarE; softmax weights / style / demod scales are
    spread across partitions with tiny K=1 matmuls (never column DMAs).
  - demod: square + reduce-over-khw (strided view) on DVE, then a
    ones-vector matmul reduces across the i partition dim; the rsqrt'd
    scale is applied per output channel in the ScalarE conv-PSUM-out copy.
    The tiny demod matmuls are emitted after the first conv groups so the
    PE never stalls waiting on DVE latency.
  - conv runs in bf16: per spatial group of <=3 row-tiles, 18 accumulated
    matmuls (i_chunk x 3 x 3) with contiguous lhsT.  x is host-padded,
    bf16-cast, and arrives as overlapping 10-row strips interleaved into
    the second half of the bank stream.
"""
import sys

if "/opt/trn_rl_repo" not in sys.path:
    sys.path.insert(0, "/opt/trn_rl_repo")

import numpy as np
import concourse.bacc as bacc
import concourse.mybir as mybir
import concourse.tile as tile
from concourse.alu_op_type import AluOpType
from concourse.bass_utils import run_bass_kernel_spmd

dt = mybir.dt
AF = mybir.ActivationFunctionType

B, F, D, KK, H, W = 8, 16, 256, 3, 64, 64
HW = H * W            # 4096
KHW = KK * KK         # 9
IC = D // 128         # 2 i-chunks
FP = F // 2           # 8 f-pair bank tiles per i-chunk
OCK = 128 * KHW       # 1152 free elems per (j) block, khw-major: col = khw*128+o
BROW = 2 * OCK        # 2304 bf16 elems per bank DMA row (f-pair)
PESPL = ((0, 512), (512, 832))   # PE mix slices per f-block
DVLO, DVHI = 832, OCK            # DVE mix slice per f-block: 320 cols keeps
                                 # the 4 MAC chains just inside the bank
                                 # window so they never gate the conv start
PW = W + 2            # padded width 66
PH_ = H + 2           # padded height 66
NS = 8                # spatial tiles (8 rows each)
SROWS = H // NS       # 8 rows per spatial tile
SN = SROWS * W        # 512 = conv matmul moving size
SCOLS = (SROWS + 2) * PW  # 660 cols per x strip (10 padded rows)
CTL = 2 * F + 2 * D   # control row: breq s0|s1, style s0|s1
CGROUPS = ((0, 1, 2), (3, 4, 5), (6, 7))

_COMPILED = None


def _build(num_devices=B):
    nc = bacc.Bacc("TRN2", target_bir_lowering=False, debug=False,
                   num_devices=num_devices)

    # x: both samples, host-padded + bf16, rows = (s, ic) blocks of 128
    x_d = nc.dram_tensor("x", [2 * D, PH_ * PW], dt.bfloat16,
                         kind="ExternalInput").ap()
    # bank: this core's o-chunk; cols 0:BROW are the mix payload, cols
    # BROW:BROW+257 (first 128 rows) carry I_128 / ones constants
    bank_d = nc.dram_tensor("bank", [IC * FP * 128, BROW + 257], dt.bfloat16,
                            kind="ExternalInput").ap()
    ctl_d = nc.dram_tensor("ctl", [1, CTL], dt.float32, kind="ExternalInput").ap()
    y_d = nc.dram_tensor("y", [2 * 128, HW], dt.float32, kind="ExternalOutput").ap()

    f32, f32r, bf16 = dt.float32, dt.float32r, dt.bfloat16

    with tile.TileContext(nc) as tc:
        with (
            tc.tile_pool(name="setup", bufs=1) as setup,
            tc.tile_pool(name="xp", bufs=1) as xp,
            tc.tile_pool(name="bankp", bufs=4) as bankp,
            tc.tile_pool(name="kern", bufs=1) as kernp,
            tc.tile_pool(name="yout", bufs=4) as youtp,
            tc.tile_pool(name="mixps", bufs=1, space="PSUM") as mixps,
            tc.tile_pool(name="convps", bufs=3, space="PSUM") as convps,
            tc.tile_pool(name="auxps", bufs=1, space="PSUM") as auxps,
        ):
            # tiny control + constants DMAs first, ahead of the bank megabytes
            ctl = setup.tile([1, CTL], f32)
            nc.sync.dma_start(ctl[:], ctl_d[:])
            consts = setup.tile([128, 257], bf16, tag="consts")
            nc.sync.dma_start(consts[:], bank_d[0:128, BROW:BROW + 257])

            bts = {}

            def issue_bank_dma(ic, fp):
                bt = bankp.tile([128, BROW], bf16, tag="bank")
                row0 = (ic * FP + fp) * 128
                nc.sync.dma_start(bt[:], bank_d[row0:row0 + 128, 0:BROW])
                bts[(ic, fp)] = bt

            xstr = {}

            def issue_x_strip(s, ic, t, tag=None):
                st = xp.tile([128, SCOLS], bf16,
                             tag=tag if tag is not None else "xs",
                             bufs=1 if tag is not None else 6)
                row0 = (s * IC + ic) * 128
                c0 = t * SROWS * PW
                nc.sync.dma_start(st[:], x_d[row0:row0 + 128, c0:c0 + SCOLS])
                xstr[(s, ic, t)] = st

            # DMA pacing: a dma_start enqueues the moment its engine reaches
            # it and all rings share the ~300GB/s core HBM budget, so strip
            # bytes issued early directly delay the bank stream that gates
            # mixing.  The three prestart strips (conv group 0's ic0 pre-run)
            # get dedicated slots and ride with the bank; the other 29 rotate
            # through a 6-deep pool in conv consumption order, so each DMA is
            # released by conv progress rather than flooding the bank window.
            for fp in range(FP):
                issue_bank_dma(0, fp)
            for t in range(3):
                issue_x_strip(0, 0, t, tag=f"xpre{t}")
            for fp in range(FP):
                issue_bank_dma(1, fp)
            for t in range(3):
                issue_x_strip(0, 1, t)
            for g in ((3, 4, 5), (6, 7)):
                for ic in range(IC):
                    for t in g:
                        issue_x_strip(0, ic, t)
            for g in ((0, 1, 2), (3, 4, 5), (6, 7)):
                for ic in range(IC):
                    for t in g:
                        issue_x_strip(1, ic, t)

            # ---------- setup: softmax weights, diag tiles, style columns ----
            ident = consts[:, 0:128]                   # I_128 (bf16-exact)
            onescol = consts[:, 128:129]
            onesrow_b = consts[0:1, 129:257]
            ones11_b = consts[0:1, 129:130]

            # softmax WITHOUT the normalization: the L2 demodulation divides
            # by the kernel norm, so a uniform scale on the weights cancels
            # exactly (the 1e-8 epsilon shift is ~1e-12 relative).  No
            # max-shift either: inputs are O(1), f32 exp overflows past ~88.
            ex = setup.tile([1, 2 * F], f32)
            nc.scalar.activation(ex[:], ctl[:, 0:2 * F], AF.Exp, bias=0.0, scale=1.0)
            wrow_b = setup.tile([1, 2 * F], bf16)
            with nc.allow_low_precision(reason="broadcast only"):
                nc.vector.tensor_copy(wrow_b[:], ex[:])
            # broadcast w across partitions with a K=1 bf16 matmul; the psum
            # lands in a mix slot (consumed before the first mix matmul)
            wbps = mixps.tile([128, 2 * F], f32, tag="mix00", name="wbps")
            nc.tensor.matmul(wbps[:], onesrow_b[:], wrow_b[:], start=True, stop=True)
            wbc = setup.tile([128, 2 * F], f32)
            nc.vector.tensor_copy(wbc[:], wbps[:])

            # per-f diagonal lhsT tiles diag(w_f) for the PE mix, per sample.
            # Built on ScalarE (scale-by-partition-column copy) to keep the
            # Vector queue clear for the mix MAC chains.
            diags = {}
            with nc.allow_low_precision(reason="bf16 diag weights; mix accumulates f32"):
                for s in range(2):
                    for f in range(F):
                        dg = setup.tile([128, 128], bf16, tag=f"diag{s}{f}")
                        nc.scalar.activation(dg[:], ident[:], AF.Copy, bias=0.0,
                                             scale=wbc[:, s * F + f:s * F + f + 1])
                        diags[(s, f)] = dg

            # pre-warm the Square/Rsqrt activation tables while ScalarE is
            # idle (a first use otherwise eats a ~1.3us ACT_TABLE_LOAD on
            # the demod / norm critical path)
            warm = setup.tile([1, 2], f32, tag="warm")
            nc.scalar.activation(warm[:, 0:1], ctl[:, 0:1], AF.Square, bias=0.0, scale=1.0)
            nc.scalar.activation(warm[:, 1:2], ctl[:, 0:1], AF.Sqrt, bias=1.0, scale=0.0)

            # (1 + style[i]) as per-partition columns via K=1 matmuls
            sty1 = setup.tile([1, 2 * D], f32)
            nc.scalar.activation(sty1[:], ctl[:, 2 * F:CTL], AF.Copy,
                                 bias=1.0, scale=1.0)
            sty1b = setup.tile([1, 2 * D], bf16)
            with nc.allow_low_precision(reason="style factors; bf16 matches bank"):
                nc.vector.tensor_copy(sty1b[:], sty1[:])
            stycols = {}
            styps = mixps.tile([128, 4], f32, tag="mix01", name="styps")
            for s in range(2):
                for ic in range(IC):
                    k = s * IC + ic
                    nc.tensor.matmul(styps[:, k:k + 1],
                                     sty1b[0:1, s * D + ic * 128:s * D + (ic + 1) * 128],
                                     ones11_b, start=True, stop=True)
                    sc = setup.tile([128, 1], f32, tag=f"sty{s}{ic}")
                    nc.scalar.activation(sc[:], styps[:, k:k + 1], AF.Copy,
                                         bias=0.0, scale=1.0)
                    stycols[(s, ic)] = sc

            # ones column for the cross-partition (i) reduction matmul
            ones_r = setup.tile([128, 1], f32r)
            nc.vector.tensor_copy(ones_r[:], onescol)
            ones12 = setup.tile([1, 2], f32)
            nc.vector.memset(ones12[:], 1.0)

            # ---------- mixing / demod / norm / conv ----------
            km = {}
            scrs = {}
            normcols = {}

            def mix_ic(ic):
                # PE: two PSUM slices per sample; DVE: MAC chain on the tail.
                # Every tile is padded to a full 2KB PSUM bank so no two
                # concurrent accumulation groups ever share a bank (hardware
                # start_tensor_calc state is not sub-bank safe).
                ps = {s: [mixps.tile([128, 512], f32, tag=f"mix{s}{k}",
                                     name=f"m{s}{k}i{ic}")[:, 0:hi - lo]
                          for k, (lo, hi) in enumerate(PESPL)]
                      for s in range(2)}
                accs = {s: (kernp.tile([128, DVHI - DVLO], f32, tag=f"acc{s}0",
                                       name=f"acc{s}0i{ic}"),
                            kernp.tile([128, DVHI - DVLO], f32, tag=f"acc{s}1",
                                       name=f"acc{s}1i{ic}"))
                       for s in range(2)}
                for fp in range(FP):
                    bt = bts[(ic, fp)]
                    for j in range(2):
                        f = 2 * fp + j
                        fo = j * OCK
                        for s in range(2):
                            for (lo, hi), p in zip(PESPL, ps[s]):
                                nc.tensor.matmul(p[:], diags[(s, f)][:],
                                                 bt[:, fo + lo:fo + hi],
                                                 start=(f == 0), stop=(f == F - 1))
                        with nc.allow_low_precision(reason="bf16 in, f32 acc"):
                            for s in range(2):
                                a = accs[s]
                                if f == 0:
                                    nc.vector.tensor_scalar(
                                        out=a[0][:], in0=bt[:, fo + DVLO:fo + DVHI],
                                        scalar1=wbc[:, s * F:s * F + 1],
                                        scalar2=None, op0=AluOpType.mult)
                                else:
                                    nc.vector.scalar_tensor_tensor(
                                        out=a[f % 2][:],
                                        in0=bt[:, fo + DVLO:fo + DVHI],
                                        scalar=wbc[:, s * F + f:s * F + f + 1],
                                        in1=a[(f + 1) % 2][:],
                                        op0=AluOpType.mult, op1=AluOpType.add)
                # style fused into the kernel copies (ScalarE, bf16 out)
                with nc.allow_low_precision(reason="conv runs bf16"):
                    for s in range(2):
                        kt = kernp.tile([128, OCK], bf16, tag=f"kern{s}{ic}",
                                        name=f"kt{s}{ic}")
                        sc = stycols[(s, ic)]
                        for (lo, hi), p in zip(PESPL, ps[s]):
                            nc.scalar.activation(kt[:, lo:hi], p[:], AF.Copy,
                                                 bias=0.0, scale=sc[:])
                        nc.scalar.activation(kt[:, DVLO:DVHI],
                                             accs[s][(F - 1) % 2][:], AF.Copy,
                                             bias=0.0, scale=sc[:])
                        km[(s, ic)] = kt
                # demod squares on ScalarE into a combined scratch (both
                # samples side by side); the khw+i reduction runs later as
                # tiny strided matmuls on the PE -- zero DVE work
                scr = kernp.tile([128, 2 * OCK], f32r, tag="sqscratch",
                                 name=f"scr{ic}")
                with nc.allow_low_precision(reason="bf16 kernel squared into f32"):
                    for s in range(2):
                        nc.scalar.activation(scr[:, s * OCK:(s + 1) * OCK],
                                             km[(s, ic)][:], AF.Square,
                                             bias=0.0, scale=1.0)
                scrs[ic] = scr

            def demod_mms(npsum, ic):
                # reduce scr over khw and the i partition dim in one shot:
                # 9 accumulated [K=128, M=1, N=(2,128)] matmuls per i-chunk
                v = scrs[ic][:, :].rearrange("p (s r o) -> p s r o", s=2, r=KHW)
                for k in range(KHW):
                    nc.tensor.matmul(npsum[:], ones_r[:], v[:, :, k, :],
                                     start=(ic == 0 and k == 0),
                                     stop=(ic == IC - 1 and k == KHW - 1))

            def norm_final(s, npsum):
                nrow = setup.tile([1, 128], f32, tag=f"nrow{s}", name=f"nrow{s}")
                nc.vector.tensor_scalar_add(nrow[:], npsum[:, s * 128:(s + 1) * 128],
                                            1e-8)
                nsq = setup.tile([1, 128], f32, tag=f"nsq{s}", name=f"nsq{s}")
                nc.scalar.activation(nsq[:], nrow[:], AF.Sqrt, bias=0.0, scale=1.0)
                nrec = setup.tile([1, 128], f32, tag=f"nrec{s}", name=f"nrec{s}")
                nc.vector.reciprocal(nrec[:], nsq[:])
                # the transpose psum rides in a (long-consumed) mix slot
                ntr = mixps.tile([128, 2], f32, tag=f"mix{s}0", name=f"ntr{s}")
                nc.tensor.matmul(ntr[:], nrec[:], ones12[:], start=True, stop=True)
                ncol = setup.tile([128, 1], f32, tag=f"ncol{s}", name=f"ncol{s}")
                nc.scalar.activation(ncol[:], ntr[:, 0:1], AF.Copy, bias=0.0, scale=1.0)
                normcols[s] = ncol

            def conv_passes(s, group, cps, ic):
                kt = km[(s, ic)]
                for k in range(KHW):
                    kh, kw = divmod(k, KK)
                    first = (ic == 0 and k == 0)
                    last = (ic == IC - 1 and k == KHW - 1)
                    for t, cp in zip(group, cps):
                        xv = xstr[(s, ic, t)][:, :].rearrange(
                            "p (r c) -> p r c", c=PW)
                        nc.tensor.matmul(
                            cp[:], kt[:, k * 128:(k + 1) * 128],
                            xv[:, kh:kh + SROWS, kw:kw + W],
                            start=first, stop=last)

            def conv_mms(s, group):
                cps = [convps.tile([128, SN], f32, tag="conv", name=f"c{s}{t}")
                       for t in group]
                for ic in range(IC):
                    conv_passes(s, group, cps, ic)
                return cps

            def conv_out(s, group, cps):
                for t, cp in zip(group, cps):
                    yt = youtp.tile([128, SN], f32, tag="y", name=f"y{s}{t}")
                    nc.scalar.activation(yt[:], cp[:], AF.Copy,
                                         bias=0.0, scale=normcols[s][:])
                    nc.gpsimd.dma_start(
                        y_d[s * 128:(s + 1) * 128, t * SN:(t + 1) * SN], yt[:])

            def conv_group(s, group):
                conv_out(s, group, conv_mms(s, group))

            npsum = auxps.tile([1, 256], f32, tag="np", name="npsum")
            mix_ic(0)
            # conv group 0's ic0 passes pre-run in the PE's idle window while
            # the second bank half streams in for mix_ic(1)
            cps0 = [convps.tile([128, SN], f32, tag="conv", name=f"c0{t}")
                    for t in CGROUPS[0]]
            conv_passes(0, CGROUPS[0], cps0, 0)
            demod_mms(npsum, 0)
            mix_ic(1)
            conv_passes(0, CGROUPS[0], cps0, 1)
            demod_mms(npsum, 1)
            norm_final(0, npsum)
            conv_out(0, CGROUPS[0], cps0)
            conv_group(0, CGROUPS[1])
            norm_final(1, npsum)
            conv_group(0, CGROUPS[2])
            for g in CGROUPS:
                conv_group(1, g)

    nc.compile()
    return nc


def _get_compiled():
    global _COMPILED
    if _COMPILED is None:
        _COMPILED = _build()
    return _COMPILED


def _make_in_maps(x, bank_request, style, bank_weight):
    bf16_np = mybir.dt.np(mybir.dt.bfloat16)
    # bank: (F, O, I, KH, KW) -> per-oc [ic, fp, i, j, khw, o_local] bf16
    A = bank_weight.astype(np.float32).reshape(FP, 2, 2, 128, IC, 128, KHW)
    #                     dims: (fp, j, oc, o_local, ic, i, khw)
    banks = []
    for oc in range(2):
        core = A[:, :, oc].transpose(3, 0, 4, 1, 5, 2).reshape(IC * FP * 128, BROW)
        #      (fp, j, o, ic, i, khw) -> (ic, fp, i, j, khw, o)
        bankT = np.zeros((IC * FP * 128, BROW + 257), dtype=np.float32)
        bankT[:, 0:BROW] = core
        bankT[0:128, BROW:BROW + 128] = np.eye(128, dtype=np.float32)
        bankT[0:128, BROW + 128] = 1.0
        bankT[0, BROW + 129:BROW + 257] = 1.0
        banks.append(np.ascontiguousarray(bankT).astype(bf16_np))

    xpad = np.zeros((B, D, PH_, PW), dtype=np.float32)
    xpad[:, :, 1:1 + H, 1:1 + W] = x.astype(np.float32).reshape(B, D, H, W)
    xpad = xpad.reshape(B, D, PH_ * PW).astype(bf16_np)

    breq = bank_request.astype(np.float32)
    sty = style.astype(np.float32).reshape(B, D)

    maps = []
    for c in range(B):
        oc = c % 2
        s0 = 2 * (c // 2)
        ctl = np.concatenate([breq[s0], breq[s0 + 1], sty[s0], sty[s0 + 1]])
        maps.append({
            "x": np.ascontiguousarray(xpad[s0:s0 + 2].reshape(2 * D, PH_ * PW)),
            "bank": banks[oc],
            "ctl": np.ascontiguousarray(ctl.reshape(1, CTL)),
        })
    return maps


def run(inputs, trace=False, **trace_kwargs):
    nc = _get_compiled()
    in_maps = _make_in_maps(inputs["x"], inputs["bank_request"],
                            inputs["style"], inputs["bank_weight"])
    # The first execution of a freshly compiled NEFF occasionally dies with
    # NRT_EXEC_UNIT_UNRECOVERABLE on this runtime; a plain retry succeeds.
    last_exc = None
    for _ in range(3):
        try:
            res = run_bass_kernel_spmd(nc, in_maps, core_ids=list(range(B)),
                                       trace=trace, **trace_kwargs)
            y = np.empty((B, D, H, W), dtype=np.float32)
            for c in range(B):
                oc = c % 2
                s0 = 2 * (c // 2)
                yc = res.results[c]["y"].reshape(2, 128, H, W)
                y[s0, oc * 128:(oc + 1) * 128] = yc[0]
                y[s0 + 1, oc * 128:(oc + 1) * 128] = yc[1]
            return y, res
        except Exception as e:  # noqa: BLE001
            last_exc = e
    raise last_exc


def kernel(x, bank_request, style, bank_weight):
    y, _ = run({"x": np.asarray(x), "bank_request": np.asarray(bank_request),
                "style": np.asarray(style), "bank_weight": np.asarray(bank_weight)})
    return y
